# revision 18
# baseline (speedup 1.0000x reference)
"""PointUpsample (3-NN inverse-distance feature interpolation) on 8 Trainium2
NeuronCores.

Problem: xyz (2,4096,3) source points with feats (2,64,4096); parent_xyz
(2,16384,3) query points. For each parent point find the 3 nearest sources
and output the inverse-distance-weighted average of their features:
out (2,64,16384).

Sharding: parents are split across the 8 cores (batch b = core//4, parent
slice = (core%4)*4096). Each core gets the full xyz/feats of its batch and
computes its 4096-parent slice independently; no collectives.

Per-core algorithm (parent tile = 128 parents, 32 tiles):
 - PE computes s = 2*p.x - |x|^2 for the tile against all 4096 sources
   (argmax of s == argmin of squared distance). The +|p|^2 term is constant
   per row so it does not affect the ranking; it is added back on the tiny
   top-3 tensor to recover d^2 for the weights.
 - ACT copies s from PSUM to SBUF; DVE max/max_index (top-8 HW instructions)
   produce the 3 nearest sources' values and indices.
 - GPSIMD indirect DMA gathers the 3 feature rows per parent from a
   transposed feature table fT[4096,64] (built once on-device via PE
   transposes).
 - DVE combines them with normalized 1/(dist+eps) weights; PE transposes the
   [128,64] result to [64,128] and it is written out.

Parent columns are fed tile-major-permuted (tile t, lane p <-> parent
32*p+t) so every per-tile access is a contiguous/fast DMA; the host undoes
the permutation when assembling the output.
"""

import numpy as np

import concourse.bass as bass
import concourse.mybir as mybir
import concourse.tile as tile
from concourse.bass_utils import run_bass_kernel_spmd
from concourse.masks import make_identity
from concourse.tile import TileContext
from concourse.vector_clock import ScopedClock

F32 = mybir.dt.float32
P = 128          # partitions / parents per tile
M = 4096         # sources per batch
C = 64           # feature channels
NPAR = 4096      # parents per core
NT = NPAR // P   # parent tiles per core
NCORES = 8
EPS = 1e-8
MCH = 1024       # psum chunk of the s row (2 banks)


def _patched_drain_and_barrier(self, tick_clock, wait_clock):
    """This walrus build rejects instructions carrying more than a couple of
    sem waits ("Too many sync wait commands"); the stock TileContext exit
    drain carries one wait per live semaphore. Split them onto NOPs."""
    probe = self.nc.sync.nop(nofuse=True, hint="tile_exit_wait_probe")
    wait_clock.add_sem_waits(probe.ins, ScopedClock({None: tick_clock.global_clock}))
    si = probe.ins.sync_info
    waits = list(si.on_wait or []) if si is not None else []
    if si is not None:
        probe.ins.sync_info.on_wait = []
    for w in waits:
        n = self.nc.sync.nop(nofuse=True, hint="tile_exit_wait")
        n.ins.sync_info = mybir.SyncInfo(on_wait=[w], on_update=[])
    self.nc.sync.drain()
    self.nc.all_engine_barrier()
    popped = self.nc._tile_sem_poison_stack.pop()
    assert popped is self._sem_poison
    self.nc.clear_and_free_semaphores(list(self.sems.allocated().values()))
    self.nc.all_engine_barrier()


TileContext._drain_and_barrier = _patched_drain_and_barrier


def _split_multi_waits(nc):
    """This walrus build supports at most ONE sync wait per instruction.
    Move excess waits onto NoOps inserted just before, on the same engine
    (sequencer executes them in order, so semantics are identical)."""
    n = 0
    for f in nc.m.functions:
        for bb in f.blocks:
            out = []
            for ins in bb.instructions:
                si = ins.sync_info
                if si is not None and si.on_wait and len(si.on_wait) > 1:
                    waits = list(si.on_wait)
                    for w in waits[:-1]:
                        nop = mybir.InstNoOp(name=f"WSPL-{n}", ins=[], outs=[])
                        n += 1
                        nop.engine = ins.engine
                        nop.sync_info = mybir.SyncInfo(on_wait=[w], on_update=[])
                        out.append(nop)
                    ins.sync_info = mybir.SyncInfo(
                        on_wait=[waits[-1]], on_update=list(si.on_update)
                    )
                out.append(ins)
            bb.instructions[:] = out
    for f in nc.m.functions:
        for bb in f.blocks:
            for ins in bb.instructions:
                si = ins.sync_info
                assert si is None or len(si.on_wait or []) <= 1, ins.name


def _build_nc(kk: int, mat_dt, repeat: int = 1, loop_reps: int | None = None):
    """Build the per-core Bass program. kk = contraction rows of the distance
    matmul; mat_dt = dtype of the lhsT/rhs distance matrices. repeat>1
    re-runs the whole tile loop (for benchmarking: overhead cancellation);
    loop_reps wraps the body in a dynamic For_i (bench only)."""
    nc = bass.Bass()
    lhsT_d = nc.dram_tensor("lhsT", [kk, NPAR], mat_dt, kind="ExternalInput")
    rhs_d = nc.dram_tensor("rhs", [kk, M], mat_dt, kind="ExternalInput")
    pn_d = nc.dram_tensor("pn", [P, NT], F32, kind="ExternalInput")
    feats_d = nc.dram_tensor("feats", [C, M], F32, kind="ExternalInput")
    out_d = nc.dram_tensor("out", [C, NPAR], F32, kind="ExternalOutput")
    fT = nc.dram_tensor("fT", [M, C], F32)  # internal: transposed feats

    AF = mybir.ActivationFunctionType

    with tile.TileContext(nc) as tc:
        with (
            tc.tile_pool(name="const", bufs=1) as const,
            tc.tile_pool(name="sbig", bufs=3) as sbig,
            tc.tile_pool(name="small", bufs=4) as small,
            tc.tile_pool(name="ps_s", bufs=3, space="PSUM") as ps_s,
            tc.tile_pool(name="ps_t", bufs=2, space="PSUM") as ps_t,
        ):
            lhsT = const.tile([kk, NPAR], mat_dt)
            rhs = const.tile([kk, M], mat_dt)
            pn = const.tile([P, NT], F32)
            feats = const.tile([C, M], F32)
            ident = const.tile([P, P], F32)

            nc.sync.dma_start(out=lhsT[:], in_=lhsT_d[:])
            nc.sync.dma_start(out=rhs[:], in_=rhs_d[:])
            nc.sync.dma_start(out=pn[:], in_=pn_d[:])
            nc.sync.dma_start(out=feats[:], in_=feats_d[:])
            make_identity(nc, ident[:])

            # Build fT = feats.T in DRAM via PE transposes.
            for i in range(M // P):
                tp = ps_t.tile([P, C], F32, tag="tp")
                nc.tensor.transpose(
                    tp[:], feats[:, i * P : (i + 1) * P], ident[:C, :C]
                )
                ftmp = small.tile([P, C], F32, tag="ftmp")
                nc.scalar.copy(ftmp[:], tp[:])
                nc.sync.dma_start(out=fT[i * P : (i + 1) * P, :], in_=ftmp[:])

            def _body():
              for t in [tt for _ in range(repeat) for tt in range(NT)]:
                s_sb = sbig.tile([P, M], F32, tag="s_sb")
                lt = lhsT[:, t * P : (t + 1) * P]
                for c4 in range(M // MCH):
                    ps = ps_s.tile([P, MCH], F32, tag="s")
                    for h in range(MCH // 512):
                        nc.tensor.matmul(
                            ps[:, h * 512 : (h + 1) * 512],
                            lt,
                            rhs[:, c4 * MCH + h * 512 : c4 * MCH + (h + 1) * 512],
                            start=True,
                            stop=True,
                        )
                    nc.scalar.copy(s_sb[:, c4 * MCH : (c4 + 1) * MCH], ps[:])

                top8 = small.tile([P, 8], F32, tag="top8")
                idx8 = small.tile([P, 8], mybir.dt.uint32, tag="idx8")
                nc.vector.max(out=top8[:], in_=s_sb[:])
                nc.vector.max_index(out=idx8[:], in_max=top8[:], in_values=s_sb[:])

                # d^2 = relu(|p|^2 - s) on the 3 nearest; then dist, weights.
                # Small ops ride on ACT (DVE is the bottleneck engine); only
                # the reciprocals (exact 1/x) and the combine stay on DVE.
                d3 = small.tile([P, 3], F32, tag="d3")
                nc.scalar.activation(
                    d3[:], top8[:, :3], AF.Relu, bias=pn[:, t : t + 1], scale=-1.0
                )
                dist = small.tile([P, 3], F32, tag="dist")
                nc.scalar.sqrt(dist[:], d3[:])
                de = small.tile([P, 3], F32, tag="de")
                nc.scalar.activation(de[:], dist[:], AF.Copy, bias=EPS)
                invd = small.tile([P, 3], F32, tag="invd")
                nc.vector.reciprocal(invd[:], de[:])
                rs = small.tile([P, 1], F32, tag="rs")
                rscr = small.tile([P, 3], F32, tag="rscr")
                nc.scalar.activation(rscr[:], invd[:], AF.Copy, accum_out=rs[:])
                rr = small.tile([P, 1], F32, tag="rr")
                nc.vector.reciprocal(rr[:], rs[:])

                # Gather the 3 feature rows per parent.
                g3 = small.tile([P, 3 * C], F32, tag="g3")
                for k in range(3):
                    nc.gpsimd.indirect_dma_start(
                        out=g3[:, k * C : (k + 1) * C],
                        out_offset=None,
                        in_=fT[:],
                        in_offset=bass.IndirectOffsetOnAxis(
                            ap=idx8[:, k : k + 1], axis=0
                        ),
                    )

                # acc[p, c] = (sum_k invd[p, k] * g3[p, k, c]) / sum_k invd
                wg = small.tile([P, 3 * C], F32, tag="wg")
                nc.vector.tensor_tensor(
                    out=wg[:].rearrange("p (k c) -> p k c", k=3),
                    in0=g3[:].rearrange("p (k c) -> p k c", k=3),
                    in1=invd[:].unsqueeze(2).broadcast_to([P, 3, C]),
                    op=mybir.AluOpType.mult,
                )
                acc = small.tile([P, C], F32, tag="acc")
                nc.vector.reduce_sum(
                    acc[:],
                    wg[:].rearrange("p (k c) -> p c k", k=3),
                    axis=mybir.AxisListType.X,
                )
                accs = small.tile([P, C], F32, tag="accs")
                nc.scalar.activation(accs[:], acc[:], AF.Copy, scale=rr[:, :1])

                # Transpose to [C, P]; batch 4 tiles per output store.
                tpo = ps_t.tile([C, P], F32, tag="tp")
                nc.tensor.transpose(tpo[:], accs[:], ident[:])
                if t % 4 == 0:
                    o4 = sbig.tile([C, 4 * P], F32, tag="o4")
                nc.scalar.copy(o4[:, (t % 4) * P : (t % 4 + 1) * P], tpo[:])
                if t % 4 == 3:
                    nc.sync.dma_start(
                        out=out_d[:, (t - 3) * P : (t + 1) * P], in_=o4[:]
                    )

            if loop_reps is None:
                _body()
            else:
                with tc.For_i(0, loop_reps, 1):
                    _body()

    _split_multi_waits(nc)
    return nc


# Tile-major parent permutation: position t*128+p  <->  parent 32*p+t.
_PERM = (np.arange(P)[None, :] * NT + np.arange(NT)[:, None]).reshape(-1)

# Distance matmul variants:
#  - "fp32": s = 2 p.x - |x|^2 as a K=4 fp32 contraction (PE fp32 runs at
#    1/4 rate, but the DVE top-k passes dominate the schedule anyway).
#  - "bf16": K=27 bf16 contraction; each fp32 operand split into three bf16
#    pieces (exact products, fp32 PSUM accumulation) -> fp32-level accuracy
#    at full bf16 PE rate.
VARIANT = "fp32"
KK = 4 if VARIANT == "fp32" else 27


def _split3(v):
    import ml_dtypes

    h = v.astype(ml_dtypes.bfloat16).astype(np.float32)
    r = (v - h).astype(np.float32)
    mi = r.astype(ml_dtypes.bfloat16).astype(np.float32)
    lo = (r - mi).astype(np.float32)
    return h, mi, lo


def _prep_core(xyz_b: np.ndarray, par_slice: np.ndarray, feats_b: np.ndarray):
    par_tm = par_slice[_PERM].astype(np.float32)  # tile-major parent order
    x = xyz_b.astype(np.float32)
    ones = np.ones(NPAR, np.float32)

    if VARIANT == "fp32":
        lhsT = np.stack([par_tm[:, 0], par_tm[:, 1], par_tm[:, 2], ones])
        x2 = ((x * x).sum(1)).astype(np.float32)
        rhs = np.stack([2.0 * x[:, 0], 2.0 * x[:, 1], 2.0 * x[:, 2], -x2])
        lhsT = lhsT.astype(np.float32)
        rhs = rhs.astype(np.float32)
    else:
        import ml_dtypes

        psp = [_split3(par_tm[:, c]) for c in range(3)]   # ph, pm, pl
        xsp = [_split3(2.0 * x[:, c]) for c in range(3)]  # of 2*x
        nsp = [_split3((x[:, c] ** 2).astype(np.float32)) for c in range(3)]
        lrows, rrows = [], []
        for c in range(3):  # small product terms first
            ph, pm, pl = psp[c]
            xh, xm, xl = xsp[c]
            for a, b in ((ph, xm), (pm, xh), (ph, xl), (pl, xh), (pm, xm)):
                lrows.append(a)
                rrows.append(b)
        for c in range(3):  # large terms interleaved
            lrows.append(psp[c][0])
            rrows.append(xsp[c][0])
            for piece in nsp[c]:
                lrows.append(ones)
                rrows.append(-piece)
        assert len(lrows) == KK
        lhsT = np.stack(lrows).astype(ml_dtypes.bfloat16)
        rhs = np.stack(rrows).astype(ml_dtypes.bfloat16)

    pn = (par_tm * par_tm).sum(1).astype(np.float32).reshape(NT, P).T.copy()
    return {
        "lhsT": np.ascontiguousarray(lhsT),
        "rhs": np.ascontiguousarray(rhs),
        "pn": np.ascontiguousarray(pn),
        "feats": np.ascontiguousarray(feats_b.astype(np.float32)),
    }


_NC_CACHE = {}


def _get_nc():
    key = (VARIANT, KK)
    if key not in _NC_CACHE:
        dt = F32 if VARIANT == "fp32" else mybir.dt.bfloat16
        _NC_CACHE[key] = _build_nc(KK, dt)
    return _NC_CACHE[key]


def kernel(xyz: np.ndarray, parent_xyz: np.ndarray, feats: np.ndarray) -> np.ndarray:
    bs, m, _ = xyz.shape
    n = parent_xyz.shape[1]
    assert (bs, m, n) == (2, M, 16384)
    nc = _get_nc()

    in_maps = []
    for core in range(NCORES):
        b = core // 4
        sl = (core % 4) * NPAR
        in_maps.append(
            _prep_core(
                np.asarray(xyz[b]),
                np.asarray(parent_xyz[b, sl : sl + NPAR]),
                np.asarray(feats[b]),
            )
        )

    res = run_bass_kernel_spmd(nc, in_maps, core_ids=list(range(NCORES)))

    out = np.empty((bs, C, n), np.float32)
    for core in range(NCORES):
        b = core // 4
        sl = (core % 4) * NPAR
        cols = sl + _PERM
        out[b][:, cols] = res.results[core]["out"]
    return out
